# revision 14
# baseline (speedup 1.0000x reference)
"""Distributed Trainium2 kernel for single-head causal AttentionBlock.

Problem: B=4, T=4096, C=1024, K=V=1024 (fp32), out = concat(x, softmax-attn read).

Sharding (8 cores, 2 per batch): core c = 2*b + par handles batch b.
  - Keys/values: core owns the 128-row key tiles with (tile % 2 == par)
    -> K/V projection split evenly across the pair, no duplicate work.
  - Queries: each core projects ALL queries of its batch (no collective,
    Q stays resident in SBUF end-to-end).
  - Each core computes UNNORMALIZED partial attention over its own keys:
      Rpart[t, v] = sum_{s in own keys, s<=t} exp(q_t . k_s / 32) * v_s
      lpart[t]    = sum_{s in own keys, s<=t} exp(q_t . k_s / 32)
  - Host merges: read = (R0 + R1) / (8*(l0 + l1)) + bv; out = concat(x, read).

All matmuls run in fp8 (e4m3) with DoubleRow perf mode (2 contraction rows
per PE pass -> 2x bf16 throughput), fp32 PSUM accumulation. Scales chosen so
every fp8 operand sits in ~[-30, 30]:
  x_hat = 16*x,  W_hat = 32*W (all three; 1/sqrt(K)=1/32 folded into Wq/bq)
  q_hat = psum_q/64 + 8*bq = 8*(q/32) ; k_hat = psum_k/64 = 8*k0
  logits_psum = q_hat . k_hat = 2048*logit  -> exp(psum/2048) on ScalarE
  v_hat = psum_v/64 = 8*v0 ; r = sum p*v_hat = 8*sum p*v0 (bf16 out)
Softmax-invariant terms dropped on device: K bias (contributes only a
per-query constant to logits) and V bias (read += bv on host).

All 8 cores run an IDENTICAL instruction stream (SPMD); only the DMA'd data
(which batch, which key rows, which diagonal masks) differs per core.
"""

import os
from contextlib import ExitStack

import numpy as np
import ml_dtypes

import concourse.bass as bass
import concourse.tile as tile
import concourse.mybir as mybir
from concourse import bacc

BF16 = mybir.dt.bfloat16
F32 = mybir.dt.float32
F8 = mybir.dt.float8e4
NP_F8 = ml_dtypes.float8_e4m3
DR = mybir.MatmulPerfMode.DoubleRow
P = 128

B, T, C = 4, 4096, 1024
KD = 1024  # key/value width
NKT = T // P          # 32 key 128-tiles per batch
NLOC = NKT // 2       # 16 local key tiles per core
NB = 8                # 512-wide query blocks
NPB = C // P          # 8 partition tiles along feature/contraction dims
NCP = NPB // 2        # 4 DoubleRow contraction pairs

LAST_RESULTS = None
_CACHE = {}


def _proj_dr(nc, ps, w_s, xs, j):
    """psum[j-tile, t] += W[:, jP:(j+1)P].T x  via 4 DoubleRow c-pair steps."""
    for cp in range(NCP):
        nc.tensor.matmul(
            ps[:],
            w_s[:, 2 * cp:2 * cp + 2, j * P:(j + 1) * P],
            xs[:, 2 * cp:2 * cp + 2, :],
            start=(cp == 0),
            stop=(cp == NCP - 1),
            perf_mode=DR,
        )


def _phase_kv(nc, tc, dram, wk_s, wv_s, kT, vv):
    """K^T and V projections over own-parity keys (4 blocks of 512)."""
    with tc.tile_pool(name="xk", bufs=2) as xkp, \
         tc.tile_pool(name="pk", bufs=2, space="PSUM") as pkp, \
         tc.tile_pool(name="pv", bufs=2, space="PSUM") as pvp:
        for blk in range(4):
            xs = xkp.tile([P, NPB, 512], F8)
            nc.sync.dma_start(out=xs[:], in_=dram["xtk"][:, blk, :, :])
            for j in range(NPB):
                ps = pkp.tile([P, 512], F32)
                _proj_dr(nc, ps, wk_s, xs, j)
                if j % 2 == 0:
                    nc.vector.tensor_scalar_mul(
                        kT[blk][:, j, :], ps[:], 1.0 / 64.0)
                else:
                    nc.scalar.mul(kT[blk][:, j, :], ps[:], 1.0 / 64.0)
            for sl in range(4):
                pv = pvp.tile([P, KD], F32)
                for vh in range(2):
                    for cp in range(NCP):
                        nc.tensor.matmul(
                            pv[:, vh * 512:(vh + 1) * 512],
                            xs[:, 2 * cp:2 * cp + 2, sl * P:(sl + 1) * P],
                            wv_s[:, 2 * cp:2 * cp + 2, vh * 512:(vh + 1) * 512],
                            start=(cp == 0),
                            stop=(cp == NCP - 1),
                            perf_mode=DR,
                        )
                if sl % 2 == 0:
                    nc.vector.tensor_scalar_mul(
                        vv[blk][:, sl, :], pv[:], 1.0 / 64.0)
                else:
                    nc.scalar.mul(vv[blk][:, sl, :], pv[:], 1.0 / 64.0)


def _phase_q(nc, tc, dram, wq_s, bq_s, qb):
    """Project all queries into resident SBUF fp8 tiles (8 blocks of 512)."""
    with tc.tile_pool(name="xq", bufs=2) as xqp, \
         tc.tile_pool(name="pq", bufs=3, space="PSUM") as pqp:
        for blk in range(NB):
            xs = xqp.tile([P, NPB, 512], F8)
            nc.sync.dma_start(out=xs[:], in_=dram["xtq"][:, blk, :, :])
            for j in range(NPB):
                ps = pqp.tile([P, 512], F32)
                _proj_dr(nc, ps, wq_s, xs, j)
                if j % 2 == 0:
                    nc.vector.tensor_scalar(
                        out=qb[blk][:, j, :],
                        in0=ps[:],
                        scalar1=1.0 / 64.0,
                        scalar2=bq_s[:, j:j + 1],
                        op0=mybir.AluOpType.mult,
                        op1=mybir.AluOpType.add,
                    )
                else:
                    nc.scalar.activation(
                        qb[blk][:, j, :], ps[:],
                        mybir.ActivationFunctionType.Identity,
                        bias=bq_s[:, j:j + 1], scale=1.0 / 64.0)


def _phase_attn(nc, tc, dram, kT, vv, qb, mk_s, ones1):
    """Per 512-query block: S^T fp8 matmuls, exp, PV/l accumulate, DMA out."""
    with tc.tile_pool(name="pt", bufs=2) as ptp, \
         tc.tile_pool(name="lsb", bufs=2) as lsbp, \
         tc.tile_pool(name="rsb", bufs=3) as rsbp, \
         tc.tile_pool(name="sp", bufs=3, space="PSUM") as spp, \
         tc.tile_pool(name="rp", bufs=2, space="PSUM") as rpp, \
         tc.tile_pool(name="lp", bufs=1, space="PSUM") as lpp:
        for jb in range(NB):
            reach = 2 * (jb + 1)  # local key tiles with any unmasked entry
            pt = ptp.tile([P, NLOC, 512], F8)
            for sl in range(reach):
                sps = spp.tile([P, 512], F32)
                for cp in range(NCP):
                    nc.tensor.matmul(
                        sps[:],
                        kT[sl // 4][:, 2 * cp:2 * cp + 2,
                                    (sl % 4) * P:(sl % 4 + 1) * P],
                        qb[jb][:, 2 * cp:2 * cp + 2, :],
                        start=(cp == 0),
                        stop=(cp == NCP - 1),
                        perf_mode=DR,
                    )
                if sl >= reach - 2:
                    nc.vector.tensor_add(
                        sps[:], sps[:], mk_s[:, sl - (reach - 2), :])
                nc.scalar.activation(
                    pt[:, sl, :], sps[:],
                    mybir.ActivationFunctionType.Exp, scale=1.0 / 2048.0)

            lps = lpp.tile([1, 512], F32)
            for sl in range(reach):
                nc.tensor.matmul(
                    lps[:], ones1[:], pt[:, sl, :],
                    start=(sl == 0), stop=(sl == reach - 1))
            l_sb = lsbp.tile([1, 512], F32)
            nc.vector.tensor_copy(l_sb[:], lps[:])
            nc.gpsimd.dma_start(
                out=dram["outl"][jb:jb + 1, :], in_=l_sb[:])

            for tj in range(4):  # 128-query subtiles; global t-tile = 4*jb+tj
                gj = 4 * jb + tj
                nsub = gj // 2 + 1  # local key tiles feeding this t-tile
                npair = nsub // 2
                rps = rpp.tile([P, KD], F32)
                for vh in range(2):
                    for i in range(npair):
                        nc.tensor.matmul(
                            rps[:, vh * 512:(vh + 1) * 512],
                            pt[:, 2 * i:2 * i + 2, tj * P:(tj + 1) * P],
                            vv[(2 * i) // 4][:, (2 * i) % 4:(2 * i) % 4 + 2,
                                             vh * 512:(vh + 1) * 512],
                            start=(i == 0),
                            stop=(i == npair - 1 and nsub % 2 == 0),
                            perf_mode=DR,
                        )
                    if nsub % 2:
                        sl = nsub - 1
                        nc.tensor.matmul(
                            rps[:, vh * 512:(vh + 1) * 512],
                            pt[:, sl, tj * P:(tj + 1) * P],
                            vv[sl // 4][:, sl % 4, vh * 512:(vh + 1) * 512],
                            start=(nsub == 1),
                            stop=True,
                        )
                r_sb = rsbp.tile([P, KD], BF16)
                nc.vector.tensor_copy(r_sb[:], rps[:])  # DVE psum->bf16
                nc.scalar.dma_start(out=dram["outr_r"][gj, :, :], in_=r_sb[:])


def _build(repeat: int = 1, phases: str = "all"):
    nc = bacc.Bacc(
        "TRN2",
        target_bir_lowering=False,
        debug=False,
        enable_asserts=False,
        num_devices=8,
    )

    # host-packed layouts: per-partition-contiguous for 4KB+ DMA descriptors
    xtq = nc.dram_tensor("xtq", [P, NB, NPB, 512], F8, kind="ExternalInput")
    xtk = nc.dram_tensor("xtk", [P, 4, NPB, 512], F8, kind="ExternalInput")
    wq = nc.dram_tensor("wq", [P, NPB, KD], F8, kind="ExternalInput")
    wk = nc.dram_tensor("wk", [P, NPB, KD], F8, kind="ExternalInput")
    wv = nc.dram_tensor("wv", [P, NPB, KD], F8, kind="ExternalInput")
    bqs = nc.dram_tensor("bqs", [P, NPB], F32, kind="ExternalInput")  # 8*bq
    mkd = nc.dram_tensor("masks", [P, 2, 512], F32, kind="ExternalInput")
    outr = nc.dram_tensor("outr", [T, KD], BF16, kind="ExternalOutput")
    outl = nc.dram_tensor("outl", [NB, 512], F32, kind="ExternalOutput")

    dram = {
        "xtq": xtq,
        "xtk": xtk,
        "outl": outl,
        "outr_r": outr.rearrange("(n p) v -> n p v", p=P),  # [32, 128, 1024]
    }

    with tile.TileContext(nc) as tc, ExitStack() as ctx:
        const = ctx.enter_context(tc.tile_pool(name="const", bufs=1))
        resid = ctx.enter_context(tc.tile_pool(name="resid", bufs=1))

        mk_s = const.tile([P, 2, 512], F32)
        nc.gpsimd.dma_start(out=mk_s[:], in_=mkd[:, :, :])
        bq_s = const.tile([P, NPB], F32)
        nc.gpsimd.dma_start(out=bq_s[:], in_=bqs[:, :])
        ones1 = const.tile([P, 1], F8)
        nc.vector.memset(ones1[:], 1.0)
        wq_s = const.tile([P, NPB, KD], F8)
        nc.gpsimd.dma_start(out=wq_s[:], in_=wq[:, :, :])
        wk_s = const.tile([P, NPB, KD], F8)
        nc.gpsimd.dma_start(out=wk_s[:], in_=wk[:, :, :])
        wv_s = const.tile([P, NPB, KD], F8)
        nc.gpsimd.dma_start(out=wv_s[:], in_=wv[:, :, :])

        # resident activations: K^T, V (own keys), Q (all, per 512-block)
        kT = [resid.tile([P, NPB, 512], F8, name=f"kT{b}", tag=f"kT{b}")
              for b in range(4)]
        vv = [resid.tile([P, 4, KD], F8, name=f"vv{b}", tag=f"vv{b}")
              for b in range(4)]
        qb = [resid.tile([P, NPB, 512], F8, name=f"qb{b}", tag=f"qb{b}")
              for b in range(NB)]

        if phases != "all":  # ablation timing: un-repeated prologue
            _phase_kv(nc, tc, dram, wk_s, wv_s, kT, vv)
            _phase_q(nc, tc, dram, wq_s, bq_s, qb)
        for _rep in range(repeat):
            if phases in ("all", "proj"):
                _phase_kv(nc, tc, dram, wk_s, wv_s, kT, vv)
                _phase_q(nc, tc, dram, wq_s, bq_s, qb)
            if phases in ("all", "attn"):
                _phase_attn(nc, tc, dram, kT, vv, qb, mk_s, ones1)

    nc.compile()
    return nc


def _get_nc():
    if "nc" not in _CACHE:
        _CACHE["nc"] = _build()
    return _CACHE["nc"]


def _get_runner(nc=None):
    """Cached jitted SPMD executor (one NEFF, 8 cores via shard_map)."""
    cache_ok = nc is None
    if cache_ok and "runner" in _CACHE:
        return _CACHE["runner"]
    import jax
    from jax.experimental.shard_map import shard_map
    from jax.sharding import Mesh, PartitionSpec
    from concourse.bass2jax import (
        _bass_exec_p,
        install_neuronx_cc_hook,
        partition_id_tensor,
    )

    if nc is None:
        nc = _get_nc()
    install_neuronx_cc_hook()
    partition_name = (
        nc.partition_id_tensor.name if nc.partition_id_tensor else None
    )
    in_names, out_names, out_avals = [], [], []
    for alloc in nc.m.functions[0].allocations:
        if not isinstance(alloc, mybir.MemoryLocationSet):
            continue
        name = alloc.memorylocations[0].name
        if alloc.kind == "ExternalInput":
            if name != partition_name:
                in_names.append(name)
        elif alloc.kind == "ExternalOutput":
            out_names.append(name)
            out_avals.append(
                jax.core.ShapedArray(
                    tuple(alloc.tensor_shape), mybir.dt.np(alloc.dtype)
                )
            )
    n_params, n_outs = len(in_names), len(out_names)
    all_in = list(in_names) + list(out_names)
    if partition_name is not None:
        all_in.append(partition_name)

    def _body(*args):
        operands = list(args)
        if partition_name is not None:
            operands.append(partition_id_tensor())
        outs = _bass_exec_p.bind(
            *operands,
            out_avals=tuple(out_avals),
            in_names=tuple(all_in),
            out_names=tuple(out_names),
            lowering_input_output_aliases=(),
            sim_require_finite=True,
            sim_require_nnan=True,
            nc=nc,
        )
        return tuple(outs)

    devices = jax.devices()[:8]
    mesh = Mesh(np.asarray(devices), ("core",))
    sharded = jax.jit(
        shard_map(
            _body,
            mesh=mesh,
            in_specs=(PartitionSpec("core"),) * (n_params + n_outs),
            out_specs=(PartitionSpec("core"),) * n_outs,
            check_rep=False,
        ),
        donate_argnums=tuple(range(n_params, n_params + n_outs)),
        keep_unused=True,
    )
    runner = (sharded, mesh, in_names, out_names, out_avals)
    if cache_ok:
        _CACHE["runner"] = runner
    return runner


def _concat_inputs(in_maps, in_names):
    return [
        np.concatenate([np.asarray(in_maps[c][nm]) for c in range(8)], axis=0)
        for nm in in_names
    ]


def _zeros_for(out_avals):
    return [
        np.zeros((8 * av.shape[0], *av.shape[1:]), av.dtype) for av in out_avals
    ]


def _run_spmd(in_maps):
    sharded, mesh, in_names, out_names, out_avals = _get_runner()
    outs = sharded(*_concat_inputs(in_maps, in_names), *_zeros_for(out_avals))
    return [
        {
            nm: np.asarray(outs[i]).reshape(8, *out_avals[i].shape)[c]
            for i, nm in enumerate(out_names)
        }
        for c in range(8)
    ]


def _make_masks(par: int) -> np.ndarray:
    # additive masks for the two diagonal-region local key tiles of each
    # 512-query block; valid (t_loc >= s_loc + d) -> 0, else -1e30.
    # layout [P, 2, 512] to match the on-device tile.
    ds = (0, 256) if par == 0 else (128, 384)
    t = np.arange(512)[None, :]
    s = np.arange(P)[:, None]
    return np.stack(
        [np.where(t >= s + d, 0.0, -1e30).astype(np.float32) for d in ds],
        axis=1,
    )


def _pack_pbt(xT_scaled: np.ndarray, nblk: int) -> np.ndarray:
    """[C, nblk*512] f32 -> fp8 [P, nblk, NPB, 512] per-partition contiguous."""
    return np.ascontiguousarray(
        xT_scaled.reshape(NPB, P, nblk, 512).transpose(1, 2, 0, 3)
    ).astype(NP_F8)


def _pack_w(w_scaled: np.ndarray) -> np.ndarray:
    """[C, KD] f32 -> fp8 [P, NPB, KD]."""
    return np.ascontiguousarray(
        w_scaled.reshape(NPB, P, KD).transpose(1, 0, 2)
    ).astype(NP_F8)


def _default_in_maps():
    rng = np.random.default_rng(0)
    in_maps = []
    for c in range(8):
        in_maps.append({
            "xtq": rng.standard_normal((P, NB, NPB, 512), np.float32).astype(NP_F8),
            "xtk": rng.standard_normal((P, 4, NPB, 512), np.float32).astype(NP_F8),
            "wq": rng.standard_normal((P, NPB, KD), np.float32).astype(NP_F8),
            "wk": rng.standard_normal((P, NPB, KD), np.float32).astype(NP_F8),
            "wv": rng.standard_normal((P, NPB, KD), np.float32).astype(NP_F8),
            "bqs": np.zeros((P, NPB), np.float32),
            "masks": _make_masks(c % 2),
        })
    return in_maps


def _prep_in_maps(minibatch, Wq, bq, Wk, bk, Wv, bv):
    minibatch = np.asarray(minibatch, dtype=np.float32)
    wq8 = _pack_w(32.0 * np.asarray(Wq, np.float32))  # 1024*(Wq/32)
    wk8 = _pack_w(32.0 * np.asarray(Wk, np.float32))
    wv8 = _pack_w(32.0 * np.asarray(Wv, np.float32))
    bqs = np.ascontiguousarray(
        (8.0 * np.asarray(bq, np.float32)).reshape(NPB, P).T)
    masks = [_make_masks(0), _make_masks(1)]

    per_batch = []
    for b in range(B):
        xT = 16.0 * minibatch[b].T                       # [C, T] f32, scaled
        xtq = _pack_pbt(xT, NB)
        xT_t = xT.reshape(C, NKT, P)
        xtk = [
            _pack_pbt(
                np.ascontiguousarray(xT_t[:, par::2, :]).reshape(C, T // 2), 4)
            for par in range(2)
        ]
        per_batch.append((xtq, xtk))

    in_maps = []
    for c in range(8):
        b, par = divmod(c, 2)
        xtq, xtk = per_batch[b]
        in_maps.append({
            "xtq": xtq,
            "xtk": xtk[par],
            "wq": wq8, "wk": wk8, "wv": wv8,
            "bqs": bqs,
            "masks": masks[par],
        })
    return in_maps


def _merge_results(minibatch, results, bv):
    minibatch = np.asarray(minibatch, dtype=np.float32)
    bv = np.asarray(bv, np.float32)
    out = np.empty((B, T, C + KD), np.float32)
    out[..., :C] = minibatch
    for b in range(B):
        r0 = np.asarray(results[2 * b]["outr"], np.float32)
        r1 = np.asarray(results[2 * b + 1]["outr"], np.float32)
        l0 = results[2 * b]["outl"].reshape(T)
        l1 = results[2 * b + 1]["outl"].reshape(T)
        out[b, :, C:] = (r0 + r1) / (8.0 * (l0 + l1))[:, None] + bv
    return out


def kernel(minibatch, Wq, bq, Wk, bk, Wv, bv):
    global LAST_RESULTS
    in_maps = _prep_in_maps(minibatch, Wq, bq, Wk, bk, Wv, bv)
    _CACHE["bench_in_maps"] = in_maps
    results = _run_spmd(in_maps)
    LAST_RESULTS = results
    return _merge_results(minibatch, results, bv)


def _time_runner(nc, n_iters: int):
    """Min wall time of one dispatch of `nc` (None -> cached repeat=1)."""
    import time
    import jax
    from jax.sharding import NamedSharding, PartitionSpec

    sharded, mesh, in_names, out_names, out_avals = _get_runner(nc)
    in_maps = _CACHE.get("bench_in_maps") or _default_in_maps()
    sh = NamedSharding(mesh, PartitionSpec("core"))
    args = [jax.device_put(a, sh) for a in _concat_inputs(in_maps, in_names)]
    jax.block_until_ready(args)

    def one():
        zeros = [jax.device_put(z, sh) for z in _zeros_for(out_avals)]
        jax.block_until_ready(zeros)
        t0 = time.perf_counter()
        outs = sharded(*args, *zeros)
        jax.block_until_ready(outs)
        return time.perf_counter() - t0

    one()  # warmup (compile on first call)
    ts = sorted(one() for _ in range(n_iters))
    print(f"  dispatch times ms: {[f'{t*1e3:.2f}' for t in ts[:10]]}")
    return ts[0]


BENCH_R2 = int(os.environ.get("KBENCH_R2", "9"))


def bench(reps: int = 7):
    """Per-iteration HW time via the repeat-slope method.

    A single dispatch through the axon tunnel carries ~25-60ms of fixed
    overhead, so wall time of one call cannot resolve the ~1ms kernel.
    Instead build the same kernel with the whole computation repeated
    R2 times and report (t_min(R2) - t_min(1)) / (R2 - 1).
    """
    n_iters = max(reps, 25)
    t1 = _time_runner(None, n_iters)
    ncR = _build(repeat=BENCH_R2)
    tR = _time_runner(ncR, n_iters)
    slope = (tR - t1) / (BENCH_R2 - 1)
    print(f"bench: t1={t1*1e3:.3f}ms tR={tR*1e3:.3f}ms (R2={BENCH_R2}) "
          f"-> per-iter {slope*1e6:.1f}us")
    return [slope]


# revision 24
# speedup vs baseline: 1.1828x; 1.1828x over previous
"""Distributed Trainium2 kernel for single-head causal AttentionBlock.

Problem: B=4, T=4096, C=1024, K=V=1024 (fp32), out = concat(x, softmax-attn read).

Sharding (8 cores, 2 per batch): core c = 2*b + par handles batch b.
  - Keys/values: core owns the 128-row key tiles with (tile % 2 == par)
    -> K/V projection split evenly across the pair, no duplicate work.
  - Queries: each core projects ALL queries of its batch (no collective,
    Q stays resident in SBUF end-to-end).
  - Each core computes UNNORMALIZED partial attention over its own keys:
      Rpart[t, v] = sum_{s in own keys, s<=t} exp(q_t . k_s / 32) * v_s
      lpart[t]    = sum_{s in own keys, s<=t} exp(q_t . k_s / 32)
  - Host merges: read = (R0 + R1) / (8*(l0 + l1)) + bv; out = concat(x, read).

All matmuls run in fp8 (e4m3) with DoubleRow perf mode (2 contraction rows
per PE pass -> 2x bf16 throughput), fp32 PSUM accumulation. Scales chosen so
every fp8 operand sits in ~[-30, 30]:
  x_hat = 16*x,  W_hat = 32*W (all three; 1/sqrt(K)=1/32 folded into Wq/bq)
  q_hat = psum_q/64 + 8*bq = 8*(q/32) ; k_hat = psum_k/64 = 8*k0
  logits_psum = q_hat . k_hat = 2048*logit  -> exp(psum/2048) on ScalarE
  v_hat = psum_v/64 = 8*v0 ; r = sum p*v_hat = 8*sum p*v0 (bf16 out)
Softmax-invariant terms dropped on device: K bias (contributes only a
per-query constant to logits) and V bias (read += bv on host).

All 8 cores run an IDENTICAL instruction stream (SPMD); only the DMA'd data
(which batch, which key rows, which diagonal masks) differs per core.
"""

import os
from contextlib import ExitStack

import numpy as np
import ml_dtypes

import concourse.bass as bass
import concourse.tile as tile
import concourse.mybir as mybir
from concourse import bacc

BF16 = mybir.dt.bfloat16
F32 = mybir.dt.float32
F8 = mybir.dt.float8e4
NP_F8 = ml_dtypes.float8_e4m3
DR = mybir.MatmulPerfMode.DoubleRow
P = 128

B, T, C = 4, 4096, 1024
KD = 1024  # key/value width
NKT = T // P          # 32 key 128-tiles per batch
NLOC = NKT // 2       # 16 local key tiles per core
NB = 8                # 512-wide query blocks
NPB = C // P          # 8 partition tiles along feature/contraction dims
NCP = NPB // 2        # 4 DoubleRow contraction pairs

LAST_RESULTS = None
_CACHE = {}


def _proj_dr(nc, ps, w_s, xs, j):
    """psum[j-tile, t] += W[:, jP:(j+1)P].T x  via 4 DoubleRow c-pair steps."""
    for cp in range(NCP):
        nc.tensor.matmul(
            ps[:],
            w_s[:, 2 * cp:2 * cp + 2, j * P:(j + 1) * P],
            xs[:, 2 * cp:2 * cp + 2, :],
            start=(cp == 0),
            stop=(cp == NCP - 1),
            perf_mode=DR,
        )


def _phase_kv(nc, tc, dram, wk_s, wv_s, kT, vv, xs_override=None):
    """K^T and V projections over own-parity keys (4 blocks of 512)."""
    with tc.tile_pool(name="xk", bufs=2) as xkp, \
         tc.tile_pool(name="pk", bufs=2, space="PSUM") as pkp, \
         tc.tile_pool(name="pv", bufs=2, space="PSUM") as pvp:
        for blk in range(4):
            if xs_override is not None:
                xs = xs_override
            else:
                xs = xkp.tile([P, NPB, 512], F8)
                nc.sync.dma_start(out=xs[:], in_=dram["xtk"][:, blk, :, :])
            for j in range(NPB):
                ps = pkp.tile([P, 512], F32)
                _proj_dr(nc, ps, wk_s, xs, j)
                if j % 2 == 0:
                    nc.vector.tensor_scalar_mul(
                        kT[blk][:, j, :], ps[:], 1.0 / 64.0)
                else:
                    nc.scalar.mul(kT[blk][:, j, :], ps[:], 1.0 / 64.0)
            for sl in range(4):
                pv = pvp.tile([P, KD], F32)
                for vh in range(2):
                    for cp in range(NCP):
                        nc.tensor.matmul(
                            pv[:, vh * 512:(vh + 1) * 512],
                            xs[:, 2 * cp:2 * cp + 2, sl * P:(sl + 1) * P],
                            wv_s[:, 2 * cp:2 * cp + 2, vh * 512:(vh + 1) * 512],
                            start=(cp == 0),
                            stop=(cp == NCP - 1),
                            perf_mode=DR,
                        )
                if sl % 2 == 0:
                    nc.vector.tensor_scalar_mul(
                        vv[blk][:, sl, :], pv[:], 1.0 / 64.0)
                else:
                    nc.scalar.mul(vv[blk][:, sl, :], pv[:], 1.0 / 64.0)


def _phase_q(nc, tc, dram, wq_s, bq_s, qb, xs_override=None):
    """Project all queries into resident SBUF fp8 tiles (8 blocks of 512)."""
    with tc.tile_pool(name="xq", bufs=2) as xqp, \
         tc.tile_pool(name="pq", bufs=3, space="PSUM") as pqp:
        for blk in range(NB):
            if xs_override is not None:
                xs = xs_override
            else:
                xs = xqp.tile([P, NPB, 512], F8)
                nc.sync.dma_start(out=xs[:], in_=dram["xtq"][:, blk, :, :])
            for j in range(NPB):
                ps = pqp.tile([P, 512], F32)
                _proj_dr(nc, ps, wq_s, xs, j)
                if j % 2 == 0:
                    nc.vector.tensor_scalar(
                        out=qb[blk][:, j, :],
                        in0=ps[:],
                        scalar1=1.0 / 64.0,
                        scalar2=bq_s[:, j:j + 1],
                        op0=mybir.AluOpType.mult,
                        op1=mybir.AluOpType.add,
                    )
                else:
                    nc.scalar.activation(
                        qb[blk][:, j, :], ps[:],
                        mybir.ActivationFunctionType.Identity,
                        bias=bq_s[:, j:j + 1], scale=1.0 / 64.0)


def _phase_attn(nc, tc, dram, kT, vv, qb, mk_s, ones1):
    """Per 512-query block: S^T fp8 matmuls, exp, PV/l accumulate, DMA out."""
    with tc.tile_pool(name="pt", bufs=2) as ptp, \
         tc.tile_pool(name="lsb", bufs=2) as lsbp, \
         tc.tile_pool(name="rsb", bufs=3) as rsbp, \
         tc.tile_pool(name="sp", bufs=3, space="PSUM") as spp, \
         tc.tile_pool(name="rp", bufs=2, space="PSUM") as rpp, \
         tc.tile_pool(name="lp", bufs=1, space="PSUM") as lpp:
        for jb in range(NB):
            reach = 2 * (jb + 1)  # local key tiles with any unmasked entry
            pt = ptp.tile([P, NLOC, 512], F8)
            for sl in range(reach):
                sps = spp.tile([P, 512], F32)
                for cp in range(NCP):
                    nc.tensor.matmul(
                        sps[:],
                        kT[sl // 4][:, 2 * cp:2 * cp + 2,
                                    (sl % 4) * P:(sl % 4 + 1) * P],
                        qb[jb][:, 2 * cp:2 * cp + 2, :],
                        start=(cp == 0),
                        stop=(cp == NCP - 1),
                        perf_mode=DR,
                    )
                if sl >= reach - 2:
                    nc.vector.tensor_add(
                        sps[:], sps[:], mk_s[:, sl - (reach - 2), :])
                nc.scalar.activation(
                    pt[:, sl, :], sps[:],
                    mybir.ActivationFunctionType.Exp, scale=1.0 / 2048.0)

            lps = lpp.tile([32, 512], F32)
            for i in range(jb + 1):
                nc.tensor.matmul(
                    lps[:], ones1[:], pt[:, 2 * i:2 * i + 2, :],
                    start=(i == 0), stop=(i == jb), perf_mode=DR)
            l_sb = lsbp.tile([1, 512], F32)
            nc.vector.tensor_copy(l_sb[:], lps[0:1, :])
            nc.gpsimd.dma_start(
                out=dram["outl"][jb:jb + 1, :], in_=l_sb[:])

            for tj in range(4):  # 128-query subtiles; global t-tile = 4*jb+tj
                gj = 4 * jb + tj
                nsub = gj // 2 + 1  # local key tiles feeding this t-tile
                npair = nsub // 2
                rps = rpp.tile([P, KD], F32)
                for vh in range(2):
                    for i in range(npair):
                        nc.tensor.matmul(
                            rps[:, vh * 512:(vh + 1) * 512],
                            pt[:, 2 * i:2 * i + 2, tj * P:(tj + 1) * P],
                            vv[(2 * i) // 4][:, (2 * i) % 4:(2 * i) % 4 + 2,
                                             vh * 512:(vh + 1) * 512],
                            start=(i == 0),
                            stop=(i == npair - 1 and nsub % 2 == 0),
                            perf_mode=DR,
                        )
                    if nsub % 2:
                        sl = nsub - 1
                        nc.tensor.matmul(
                            rps[:, vh * 512:(vh + 1) * 512],
                            pt[:, sl, tj * P:(tj + 1) * P],
                            vv[sl // 4][:, sl % 4, vh * 512:(vh + 1) * 512],
                            start=(nsub == 1),
                            stop=True,
                        )
                r_sb = rsbp.tile([P, KD], BF16)
                nc.vector.tensor_copy(r_sb[:], rps[:])  # DVE psum->bf16
                nc.scalar.dma_start(out=dram["outr_r"][gj, :, :], in_=r_sb[:])


def _build(repeat: int = 1, phases: str = "all"):
    nc = bacc.Bacc(
        "TRN2",
        target_bir_lowering=False,
        debug=False,
        enable_asserts=False,
        num_devices=8,
    )

    # host-packed layouts: per-partition-contiguous for 4KB+ DMA descriptors
    xtq = nc.dram_tensor("xtq", [P, NB, NPB, 512], F8, kind="ExternalInput")
    xtk = nc.dram_tensor("xtk", [P, 4, NPB, 512], F8, kind="ExternalInput")
    wq = nc.dram_tensor("wq", [P, NPB, KD], F8, kind="ExternalInput")
    wk = nc.dram_tensor("wk", [P, NPB, KD], F8, kind="ExternalInput")
    wv = nc.dram_tensor("wv", [P, NPB, KD], F8, kind="ExternalInput")
    bqs = nc.dram_tensor("bqs", [P, NPB], F32, kind="ExternalInput")  # 8*bq
    mkd = nc.dram_tensor("masks", [P, 2, 512], F32, kind="ExternalInput")
    outr = nc.dram_tensor("outr", [T, KD], BF16, kind="ExternalOutput")
    outl = nc.dram_tensor("outl", [NB, 512], F32, kind="ExternalOutput")

    dram = {
        "xtq": xtq,
        "xtk": xtk,
        "outl": outl,
        "outr_r": outr.rearrange("(n p) v -> n p v", p=P),  # [32, 128, 1024]
    }

    with tile.TileContext(nc) as tc, ExitStack() as ctx:
        const = ctx.enter_context(tc.tile_pool(name="const", bufs=1))
        resid = ctx.enter_context(tc.tile_pool(name="resid", bufs=1))

        mk_s = const.tile([P, 2, 512], F32)
        nc.gpsimd.dma_start(out=mk_s[:], in_=mkd[:, :, :])
        bq_s = const.tile([P, NPB], F32)
        nc.gpsimd.dma_start(out=bq_s[:], in_=bqs[:, :])
        ones1 = const.tile([P, 2, 32], F8)
        nc.vector.memset(ones1[:], 1.0)
        wq_s = const.tile([P, NPB, KD], F8)
        nc.gpsimd.dma_start(out=wq_s[:], in_=wq[:, :, :])
        wk_s = const.tile([P, NPB, KD], F8)
        nc.gpsimd.dma_start(out=wk_s[:], in_=wk[:, :, :])
        wv_s = const.tile([P, NPB, KD], F8)
        nc.gpsimd.dma_start(out=wv_s[:], in_=wv[:, :, :])

        # resident activations: K^T, V (own keys), Q (all, per 512-block)
        kT = [resid.tile([P, NPB, 512], F8, name=f"kT{b}", tag=f"kT{b}")
              for b in range(4)]
        vv = [resid.tile([P, 4, KD], F8, name=f"vv{b}", tag=f"vv{b}")
              for b in range(4)]
        qb = [resid.tile([P, NPB, 512], F8, name=f"qb{b}", tag=f"qb{b}")
              for b in range(NB)]

        if phases != "all":  # ablation timing: un-repeated prologue
            _phase_kv(nc, tc, dram, wk_s, wv_s, kT, vv)
            _phase_q(nc, tc, dram, wq_s, bq_s, qb)
        for _rep in range(repeat):
            if phases == "projdma":  # x DMA stream only
                with tc.tile_pool(name="xd", bufs=2) as xdp:
                    for blk in range(4):
                        xs = xdp.tile([P, NPB, 512], F8, name="xsd", tag="xsd")
                        nc.sync.dma_start(
                            out=xs[:], in_=dram["xtk"][:, blk, :, :])
                        nc.vector.tensor_copy(kT[blk][:, 0, 0:128], xs[:, 0, 0:128])
                    for blk in range(NB):
                        xs = xdp.tile([P, NPB, 512], F8, name="xsd", tag="xsd")
                        nc.sync.dma_start(
                            out=xs[:], in_=dram["xtq"][:, blk, :, :])
                        nc.vector.tensor_copy(qb[blk][:, 0, 0:128], xs[:, 0, 0:128])
            if phases == "mmsame":  # same-lhsT repeat: is Ldweights elided?
                xs0 = const.tile([P, NPB, 512], F8, name=f"xq{_rep}",
                                 tag=f"xq{_rep}")
                nc.sync.dma_start(out=xs0[:], in_=dram["xtk"][:, 0, :, :])
                with tc.tile_pool(name="pm", bufs=1, space="PSUM") as pmp:
                    pm = pmp.tile([P, 512], F32)
                    for i in range(512):
                        nc.tensor.matmul(
                            pm[:],
                            wq_s[:, 0:2, 0:P],
                            xs0[:, 0:2, :],
                            start=(i == 0), stop=(i == 511), perf_mode=DR)
                    nc.vector.tensor_scalar_mul(
                        qb[0][:, 0, :], pm[:], 1.0 / 64.0)
            if phases == "mm512":  # raw PE DoubleRow throughput
                xs0 = const.tile([P, NPB, 512], F8, name=f"xm{_rep}",
                                 tag=f"xm{_rep}")
                nc.sync.dma_start(out=xs0[:], in_=dram["xtk"][:, 0, :, :])
                with tc.tile_pool(name="pm", bufs=1, space="PSUM") as pmp:
                    pm = pmp.tile([P, 512], F32)
                    for i in range(512):
                        cp = i % 4
                        j = (i // 4) % 8
                        nc.tensor.matmul(
                            pm[:],
                            wq_s[:, 2 * cp:2 * cp + 2, j * P:(j + 1) * P],
                            xs0[:, 2 * cp:2 * cp + 2, :],
                            start=(i == 0), stop=(i == 511), perf_mode=DR)
                    nc.vector.tensor_scalar_mul(
                        qb[0][:, 0, :], pm[:], 1.0 / 64.0)
            if phases == "projnodma":
                xs0 = const.tile([P, NPB, 512], F8, name=f"xs0_{_rep}",
                                 tag=f"xs0_{_rep}")
                nc.sync.dma_start(out=xs0[:], in_=dram["xtk"][:, 0, :, :])
                _phase_kv(nc, tc, dram, wk_s, wv_s, kT, vv, xs_override=xs0)
                _phase_q(nc, tc, dram, wq_s, bq_s, qb, xs_override=xs0)
            if phases in ("all", "proj"):
                _phase_kv(nc, tc, dram, wk_s, wv_s, kT, vv)
                _phase_q(nc, tc, dram, wq_s, bq_s, qb)
            if phases in ("all", "attn"):
                _phase_attn(nc, tc, dram, kT, vv, qb, mk_s, ones1)

    nc.compile()
    return nc


def _get_nc():
    if "nc" not in _CACHE:
        _CACHE["nc"] = _build()
    return _CACHE["nc"]


def _get_runner(nc=None):
    """Cached jitted SPMD executor (one NEFF, 8 cores via shard_map)."""
    cache_ok = nc is None
    if cache_ok and "runner" in _CACHE:
        return _CACHE["runner"]
    import jax
    from jax.experimental.shard_map import shard_map
    from jax.sharding import Mesh, PartitionSpec
    from concourse.bass2jax import (
        _bass_exec_p,
        install_neuronx_cc_hook,
        partition_id_tensor,
    )

    if nc is None:
        nc = _get_nc()
    install_neuronx_cc_hook()
    partition_name = (
        nc.partition_id_tensor.name if nc.partition_id_tensor else None
    )
    in_names, out_names, out_avals = [], [], []
    for alloc in nc.m.functions[0].allocations:
        if not isinstance(alloc, mybir.MemoryLocationSet):
            continue
        name = alloc.memorylocations[0].name
        if alloc.kind == "ExternalInput":
            if name != partition_name:
                in_names.append(name)
        elif alloc.kind == "ExternalOutput":
            out_names.append(name)
            out_avals.append(
                jax.core.ShapedArray(
                    tuple(alloc.tensor_shape), mybir.dt.np(alloc.dtype)
                )
            )
    n_params, n_outs = len(in_names), len(out_names)
    all_in = list(in_names) + list(out_names)
    if partition_name is not None:
        all_in.append(partition_name)

    def _body(*args):
        operands = list(args)
        if partition_name is not None:
            operands.append(partition_id_tensor())
        outs = _bass_exec_p.bind(
            *operands,
            out_avals=tuple(out_avals),
            in_names=tuple(all_in),
            out_names=tuple(out_names),
            lowering_input_output_aliases=(),
            sim_require_finite=True,
            sim_require_nnan=True,
            nc=nc,
        )
        return tuple(outs)

    devices = jax.devices()[:8]
    mesh = Mesh(np.asarray(devices), ("core",))
    sharded = jax.jit(
        shard_map(
            _body,
            mesh=mesh,
            in_specs=(PartitionSpec("core"),) * (n_params + n_outs),
            out_specs=(PartitionSpec("core"),) * n_outs,
            check_rep=False,
        ),
        donate_argnums=tuple(range(n_params, n_params + n_outs)),
        keep_unused=True,
    )
    runner = (sharded, mesh, in_names, out_names, out_avals)
    if cache_ok:
        _CACHE["runner"] = runner
    return runner


def _concat_inputs(in_maps, in_names):
    return [
        np.concatenate([np.asarray(in_maps[c][nm]) for c in range(8)], axis=0)
        for nm in in_names
    ]


def _zeros_for(out_avals):
    return [
        np.zeros((8 * av.shape[0], *av.shape[1:]), av.dtype) for av in out_avals
    ]


def _run_spmd(in_maps):
    sharded, mesh, in_names, out_names, out_avals = _get_runner()
    outs = sharded(*_concat_inputs(in_maps, in_names), *_zeros_for(out_avals))
    return [
        {
            nm: np.asarray(outs[i]).reshape(8, *out_avals[i].shape)[c]
            for i, nm in enumerate(out_names)
        }
        for c in range(8)
    ]


def _make_masks(par: int) -> np.ndarray:
    # additive masks for the two diagonal-region local key tiles of each
    # 512-query block; valid (t_loc >= s_loc + d) -> 0, else -1e30.
    # layout [P, 2, 512] to match the on-device tile.
    ds = (0, 256) if par == 0 else (128, 384)
    t = np.arange(512)[None, :]
    s = np.arange(P)[:, None]
    return np.stack(
        [np.where(t >= s + d, 0.0, -1e30).astype(np.float32) for d in ds],
        axis=1,
    )


def _pack_pbt(xT_scaled: np.ndarray, nblk: int) -> np.ndarray:
    """[C, nblk*512] f32 -> fp8 [P, nblk, NPB, 512] per-partition contiguous."""
    return np.ascontiguousarray(
        xT_scaled.reshape(NPB, P, nblk, 512).transpose(1, 2, 0, 3)
    ).astype(NP_F8)


def _pack_w(w_scaled: np.ndarray) -> np.ndarray:
    """[C, KD] f32 -> fp8 [P, NPB, KD]."""
    return np.ascontiguousarray(
        w_scaled.reshape(NPB, P, KD).transpose(1, 0, 2)
    ).astype(NP_F8)


def _default_in_maps():
    rng = np.random.default_rng(0)
    in_maps = []
    for c in range(8):
        in_maps.append({
            "xtq": rng.standard_normal((P, NB, NPB, 512), np.float32).astype(NP_F8),
            "xtk": rng.standard_normal((P, 4, NPB, 512), np.float32).astype(NP_F8),
            "wq": rng.standard_normal((P, NPB, KD), np.float32).astype(NP_F8),
            "wk": rng.standard_normal((P, NPB, KD), np.float32).astype(NP_F8),
            "wv": rng.standard_normal((P, NPB, KD), np.float32).astype(NP_F8),
            "bqs": np.zeros((P, NPB), np.float32),
            "masks": _make_masks(c % 2),
        })
    return in_maps


def _prep_in_maps(minibatch, Wq, bq, Wk, bk, Wv, bv):
    minibatch = np.asarray(minibatch, dtype=np.float32)
    wq8 = _pack_w(32.0 * np.asarray(Wq, np.float32))  # 1024*(Wq/32)
    wk8 = _pack_w(32.0 * np.asarray(Wk, np.float32))
    wv8 = _pack_w(32.0 * np.asarray(Wv, np.float32))
    bqs = np.ascontiguousarray(
        (8.0 * np.asarray(bq, np.float32)).reshape(NPB, P).T)
    masks = [_make_masks(0), _make_masks(1)]

    per_batch = []
    for b in range(B):
        xT = 16.0 * minibatch[b].T                       # [C, T] f32, scaled
        xtq = _pack_pbt(xT, NB)
        xT_t = xT.reshape(C, NKT, P)
        xtk = [
            _pack_pbt(
                np.ascontiguousarray(xT_t[:, par::2, :]).reshape(C, T // 2), 4)
            for par in range(2)
        ]
        per_batch.append((xtq, xtk))

    in_maps = []
    for c in range(8):
        b, par = divmod(c, 2)
        xtq, xtk = per_batch[b]
        in_maps.append({
            "xtq": xtq,
            "xtk": xtk[par],
            "wq": wq8, "wk": wk8, "wv": wv8,
            "bqs": bqs,
            "masks": masks[par],
        })
    return in_maps


def _merge_results(minibatch, results, bv):
    minibatch = np.asarray(minibatch, dtype=np.float32)
    bv = np.asarray(bv, np.float32)
    out = np.empty((B, T, C + KD), np.float32)
    out[..., :C] = minibatch
    for b in range(B):
        r0 = np.asarray(results[2 * b]["outr"], np.float32)
        r1 = np.asarray(results[2 * b + 1]["outr"], np.float32)
        l0 = results[2 * b]["outl"].reshape(T)
        l1 = results[2 * b + 1]["outl"].reshape(T)
        out[b, :, C:] = (r0 + r1) / (8.0 * (l0 + l1))[:, None] + bv
    return out


def kernel(minibatch, Wq, bq, Wk, bk, Wv, bv):
    global LAST_RESULTS
    in_maps = _prep_in_maps(minibatch, Wq, bq, Wk, bk, Wv, bv)
    _CACHE["bench_in_maps"] = in_maps
    results = _run_spmd(in_maps)
    LAST_RESULTS = results
    return _merge_results(minibatch, results, bv)


def _make_timer(nc):
    """Return a zero-arg callable timing one dispatch of `nc`."""
    import time
    import jax
    from jax.sharding import NamedSharding, PartitionSpec

    sharded, mesh, in_names, out_names, out_avals = _get_runner(nc)
    in_maps = _CACHE.get("bench_in_maps") or _default_in_maps()
    sh = NamedSharding(mesh, PartitionSpec("core"))
    args = [jax.device_put(a, sh) for a in _concat_inputs(in_maps, in_names)]
    jax.block_until_ready(args)

    def one():
        zeros = [jax.device_put(z, sh) for z in _zeros_for(out_avals)]
        jax.block_until_ready(zeros)
        t0 = time.perf_counter()
        outs = sharded(*args, *zeros)
        jax.block_until_ready(outs)
        return time.perf_counter() - t0

    return one


def _slope_pair(nc1, ncR, r2: int, n_rounds: int = 100):
    """Interleaved min-wall-time slope: (min tR - min t1) / (r2 - 1).

    Dispatch overhead through the axon tunnel is bimodal (~27ms / ~67ms
    modes); interleaving the two runners lets both sample the fast mode
    and the min-difference cancels the fixed overhead.
    """
    t_a, t_b = _make_timer(nc1), _make_timer(ncR)
    t_a(), t_b()  # warmup / compile
    m1 = tR = float("inf")
    s1, sR = [], []
    for _ in range(n_rounds):
        s1.append(t_a())
        sR.append(t_b())
    m1, mR = min(s1), min(sR)
    print(f"  t1 mins ms: {[f'{t*1e3:.2f}' for t in sorted(s1)[:6]]}")
    print(f"  tR mins ms: {[f'{t*1e3:.2f}' for t in sorted(sR)[:6]]}")
    slope = (mR - m1) / (r2 - 1)
    print(f"  slope: t1={m1*1e3:.3f}ms tR={mR*1e3:.3f}ms (R2={r2}) "
          f"-> per-iter {slope*1e6:.1f}us")
    return slope


BENCH_R2 = int(os.environ.get("KBENCH_R2", "9"))


def bench(reps: int = 7):
    """Per-iteration HW time via the repeat-slope method.

    A single dispatch through the axon tunnel carries ~25-60ms of fixed
    overhead, so wall time of one call cannot resolve the ~1ms kernel.
    Instead build the same kernel with the whole computation repeated
    R2 times and report (t_min(R2) - t_min(1)) / (R2 - 1).
    """
    n_rounds = max(reps, 30)
    ncR = _build(repeat=BENCH_R2)
    slope = _slope_pair(None, ncR, BENCH_R2, n_rounds)
    return [slope]
